# revision 26
# baseline (speedup 1.0000x reference)
"""Bass/Trainium2 kernel for nn_BagModel (segment_reduce).

Model: h = relu(x @ W1 + b1); per-bag mean of h over sorted ids;
out = means @ W2 + b2.   x:[500000,128] f32, ids:[500000] sorted int64,
W1:[128,256], W2:[256,64], B=10000 bags.

Strategy (8 cores, data-parallel over rows):
- Host: shard rows 62500/core; xT bf16 [128, rows]; group tiles into
  windows spanning < 32 bags (shared group structure across cores, all
  even sizes via padding T to 490); onehot plane [128, T, 32] fp8e4
  (row r of tile t: col rel(r) = bag - window_base).
- Device: per tile h_psum = xT_t.T @ W1 (PE bf16, 256-col stream);
  relu per QUAD of 4 tiles -> SBUF fp8e4 (ACT/DVE alternating 6:5);
  per PAIR one fp8 DoubleRow matmul oh[128,2,32].T @ h2[128,2,256]
  accumulating sums[32,256] in PSUM over the window (dst partition 0
  required by ISA for DoubleRow).
- Window end: copy PSUM->SBUF (ACT/DVE alternating), DMA out
  [NG, 32, 256] f32 partials.
- Host: overlap-add windows into [10000,256], divide by counts,
  means @ W2 + b2 (host GEMM).
"""

import numpy as np
import ml_dtypes
from contextlib import ExitStack

from concourse import bass, tile
from concourse.bass import mybir
from concourse.bass_utils import run_bass_kernel_spmd

N_CORES = 8
N_FULL, D, H, O, B = 500000, 128, 256, 64, 10000
P = 128  # partitions / tile rows
WW = 64  # bag-window width (onehot width)

F32 = mybir.dt.float32
BF16 = mybir.dt.bfloat16
FP8 = mybir.dt.float8e4
BF = ml_dtypes.bfloat16
E4 = ml_dtypes.float8_e4m3

DR = mybir.MatmulPerfMode.DoubleRow
Relu = mybir.ActivationFunctionType.Relu


def build_nc(T, group_sizes, b1_nonzero, relu_pattern=(7, 6), split_waits=True):
    """One-core program; SPMD-run on all 8 cores with different data.

    T is padded so that every group size is even. Tiles are processed in
    quads (4) + leftover pairs; relu batched per quad.
    """
    NG = len(group_sizes)
    nc = bass.Bass()

    # chunk schedule: ramp up so PE starts early and never outruns DMA
    CH = 32
    chunk_starts = [0, 4, 12, 28]
    while chunk_starts[-1] + CH < T:
        chunk_starts.append(chunk_starts[-1] + CH)
    chunk_of = {}
    for ci, cs in enumerate(chunk_starts):
        ce = chunk_starts[ci + 1] if ci + 1 < len(chunk_starts) else T
        for t in range(cs, ce):
            chunk_of[t] = (ci, cs, ce - cs)

    xt_d = nc.dram_tensor("xt", [P, T * P], BF16, kind="ExternalInput")
    oh_d = nc.dram_tensor("oh", [P, T, WW], FP8, kind="ExternalInput")
    w1_d = nc.dram_tensor("w1", [D, H], BF16, kind="ExternalInput")
    if b1_nonzero:
        b1_d = nc.dram_tensor("b1", [1, H], BF16, kind="ExternalInput")
    out_d = nc.dram_tensor("out_parts", [NG, WW, H], F32, kind="ExternalOutput")

    with tile.TileContext(nc) as tc, ExitStack() as ctx:
        consts = ctx.enter_context(tc.tile_pool(name="consts", bufs=1))
        w1_sb = consts.tile([D, H], BF16)
        if b1_nonzero:
            b1_sb = consts.tile([1, H], BF16)
            ones1_sb = consts.tile([1, P], BF16)
            nc.sync.dma_start(b1_sb[:], b1_d[:])
            nc.gpsimd.memset(ones1_sb[:], 1.0)

        xt_pool = ctx.enter_context(tc.tile_pool(name="xt", bufs=4))
        oh_pool = ctx.enter_context(tc.tile_pool(name="oh", bufs=4))
        h_pool = ctx.enter_context(tc.tile_pool(name="h", bufs=5))
        fl_pool = ctx.enter_context(tc.tile_pool(name="fl", bufs=8))
        hps_pool = ctx.enter_context(
            tc.tile_pool(name="hps", bufs=3, space=bass.MemorySpace.PSUM)
        )
        sps_pool = ctx.enter_context(
            tc.tile_pool(name="sps", bufs=2, space=bass.MemorySpace.PSUM)
        )

        ra, rd = relu_pattern
        rmod = ra + rd

        xt_chunk = oh_chunk = None

        def chunk_loads(t, first=False):
            ci, cs, cw = chunk_of[t]
            if t != cs:
                return
            nonlocal xt_chunk, oh_chunk
            xt_chunk = xt_pool.tile([P, CH * P], BF16)
            nc.sync.dma_start(
                xt_chunk[:, 0 : cw * P], xt_d[:, cs * P : (cs + cw) * P]
            )
            oh_chunk = oh_pool.tile([P, CH, WW], FP8)
            nc.sync.dma_start(oh_chunk[:, 0:cw, :], oh_d[:, cs : cs + cw, :])
            if first:
                # consts on a parallel queue so they overlap the first chunk
                nc.scalar.dma_start(w1_sb[:], w1_d[:])

        def h_matmul(h_ps_slice, t):
            ci, cs, cw = chunk_of[t]
            lhs = xt_chunk[:, (t - cs) * P : (t - cs + 1) * P]
            if b1_nonzero:
                nc.tensor.matmul(h_ps_slice, lhs, w1_sb[:], start=True, stop=False)
                nc.tensor.matmul(
                    h_ps_slice, ones1_sb[:], b1_sb[:], start=False, stop=True
                )
            else:
                nc.tensor.matmul(h_ps_slice, lhs, w1_sb[:], start=True, stop=True)

        def relu(dst, src, qidx):
            if qidx % rmod < ra:
                nc.scalar.activation(dst, src, Relu)
            else:
                nc.vector.tensor_scalar_max(dst, src, 0.0)

        def seg_matmul(sums_ps, h2, hoff, t, oh_c, start, stop):
            ci, cs, cw = chunk_of[t]
            nc.tensor.matmul(
                sums_ps[:, :],
                oh_c[:, t - cs : t - cs + 2, :],
                h2[:, hoff : hoff + 2, :],
                start=start,
                stop=stop,
                perf_mode=DR,
                tile_position=(0, 0),
            )

        t = 0
        qidx = 0  # relu engine selector
        pending_flush = None  # (g, sums_ps) deferred past next window's 1st round
        pending_dr = []  # seg matmuls deferred one round so PE never waits relu

        def emit_flush(relu_was_act):
            nonlocal pending_flush
            if pending_flush is None:
                return
            fg, fps = pending_flush
            pending_flush = None
            sums_sb = fl_pool.tile([WW, H], F32, tag="sums_sb")
            # flush on the engine the current relu is NOT using
            if relu_was_act:
                nc.vector.tensor_copy(sums_sb[:], fps[:])
            else:
                nc.scalar.copy(sums_sb[:], fps[:])
            nc.gpsimd.dma_start(out_d[fg], sums_sb[:])

        def emit_pending_dr():
            for args in pending_dr:
                seg_matmul(*args)
            pending_dr.clear()

        for g, gs in enumerate(group_sizes):
            assert gs % 2 == 0
            sums_ps = sps_pool.tile([WW, H], F32)
            done = 0
            while done < gs:
                qw = min(4, gs - done)  # 4 or 2 tiles this round
                chunk_loads(t, first=(t == 0))
                # tiles of one round never straddle a chunk: starts stay even
                # and chunk starts are multiples of 4
                if t + qw - 1 >= chunk_of[t][1] + chunk_of[t][2]:
                    qw = 2  # don't cross into the next chunk with a quad
                h_ps = hps_pool.tile([P, 4, H], F32, tag="hps")
                for c in range(qw):
                    h_matmul(h_ps[:, c, :], t + c)
                h2 = h_pool.tile([P, 4, H], FP8, tag="h2")
                relu(h2[:, 0:qw, :], h_ps[:, 0:qw, :], qidx)
                emit_pending_dr()  # prior round's seg MMs: relu already landed
                emit_flush(qidx % rmod < ra)  # prior window flush, other engine
                qidx += 1
                oh_c = oh_chunk  # bind current chunk tile for deferred emission
                for c in range(0, qw, 2):
                    pending_dr.append(
                        (
                            sums_ps,
                            h2,
                            c,
                            t + c,
                            oh_c,
                            done + c == 0,
                            done + c + 2 == gs,
                        )
                    )
                done += qw
                t += qw
            pending_flush = (g, sums_ps)
        emit_pending_dr()
        emit_flush(True)

    if split_waits:
        _split_excess_waits(nc)
    return nc


# walrus codegen rejects instructions whose inline sync-wait list exceeds the
# ISA struct's slots. Move excess waits to standalone EventSemaphore ops on the
# same engine right before the instruction — same-engine FIFO keeps semantics.
_WAIT_LIMITS = {
    "InstTensorTensor": 1,
    "InstTensorScalarPtr": 1,
    "InstTensorScalar": 1,
    "InstTensorCopy": 1,
    "InstTensorReduce": 1,
    "InstCopy": 1,
    "InstActivation": 1,
    "InstMatmult": 1,
    "InstLdweights": 1,
    "InstMemset": 1,
    "InstDMACopy": 1,
    "InstDrain": 1,
    "InstNoOp": 1,
    "InstEventSemaphore": 1,
}


def _split_excess_waits(nc):
    for bb in nc.main_func.blocks:
        new_list = []
        for ins in bb.instructions:
            limit = _WAIT_LIMITS.get(type(ins).__name__)
            si = ins.sync_info
            if limit is not None and si is not None and len(si.on_wait) > limit:
                waits = list(si.on_wait)
                excess, keep = waits[: len(waits) - limit], waits[len(waits) - limit :]
                for w in excess:
                    ev = mybir.InstEventSemaphore(
                        name=nc.get_next_instruction_name(),
                        engine=ins.engine,
                        ins=[],
                        outs=[],
                        sync_info=mybir.SyncInfo(on_wait=[w], on_update=[]),
                    )
                    new_list.append(ev)
                ins.sync_info = mybir.SyncInfo(on_wait=keep, on_update=list(si.on_update))
            new_list.append(ins)
        bb.instructions[:] = new_list


def choose_group_size(ids, rows_per_core, T, n_cores):
    """Largest even G (tiles/group) s.t. every group's bag span < WW on
    every core. T is the padded tile count; padding rows are ignored."""
    for G in (26, 24, 22, 20, 18, 16, 14, 12, 10, 8, 6, 4, 2):
        ok = True
        for k in range(n_cores):
            ids_k = ids[k * rows_per_core : (k + 1) * rows_per_core]
            g = 0
            while g * G < T and ok:
                s = g * G * P
                e = min((g * G + G) * P, rows_per_core)
                if s < rows_per_core:
                    if ids_k[e - 1] - ids_k[s] >= WW:
                        ok = False
                g += 1
            if not ok:
                break
        if ok:
            return G
    raise ValueError("no group size satisfies bag-span < WW")


def prepare_core_inputs(x, ids, W1, b1, rows_per_core, T, group_sizes, n_cores):
    """Returns (in_maps, bases[n_cores, NG], b1_nonzero)."""
    NG = len(group_sizes)
    rpad = T * P
    w1_bf = np.ascontiguousarray(W1.astype(BF))
    b1_nonzero = bool(np.any(b1))

    in_maps = []
    bases = np.zeros((n_cores, NG), np.int64)
    for k in range(n_cores):
        ids_k = ids[k * rows_per_core : (k + 1) * rows_per_core]
        x_k = x[k * rows_per_core : (k + 1) * rows_per_core]
        rel = np.full(rpad, -1, np.int64)
        t0 = 0
        for g, gs in enumerate(group_sizes):
            s = t0 * P
            e = min(s + gs * P, rows_per_core)
            base = int(ids_k[min(s, rows_per_core - 1)])
            bases[k, g] = base
            if s < rows_per_core:
                r = ids_k[s:e].astype(np.int64) - base
                assert r.min() >= 0 and r.max() < WW, (
                    f"bag span violation core {k} group {g}: {r.min()}..{r.max()}"
                )
                rel[s:e] = r
            t0 += gs
        # onehot plane [P, T, WW] fp8: row r=(t,p) sets col rel[r]
        oh = np.zeros((P, T, WW), E4)
        rr = np.arange(rpad)
        valid = rel >= 0
        oh[rr[valid] % P, rr[valid] // P, rel[valid]] = 1
        xt = np.zeros((P, rpad), BF)
        xt[:, :rows_per_core] = x_k.astype(BF).T
        m = {"xt": xt, "oh": oh, "w1": w1_bf}
        if b1_nonzero:
            m["b1"] = np.ascontiguousarray(b1.astype(BF).reshape(1, H))
        in_maps.append(m)
    return in_maps, bases, b1_nonzero


def merge_outputs(results, bases, ids, W2, b2, group_sizes, n_cores, num_bags):
    NG = len(group_sizes)
    acc = np.zeros((num_bags + WW, H), np.float32)
    for k in range(n_cores):
        parts = np.asarray(results[k]["out_parts"], np.float32)  # [NG, WW, H]
        for g in range(NG):
            acc[bases[k, g] : bases[k, g] + WW] += parts[g]
    counts = np.bincount(ids.astype(np.int64), minlength=num_bags)[:num_bags]
    means = acc[:num_bags] / np.maximum(counts, 1.0)[:, None]
    out = means @ W2.astype(np.float32) + b2.astype(np.float32)
    return out.astype(np.float32)


def kernel_traced(x, ids, W1, b1, W2, b2, trace=False, **spmd_kwargs):
    x = np.asarray(x)
    ids = np.asarray(ids).astype(np.int64)
    W1 = np.asarray(W1)
    b1 = np.asarray(b1)
    W2 = np.asarray(W2)
    b2 = np.asarray(b2)

    rows = N_FULL // N_CORES
    T = (rows + P - 1) // P
    if T % 2:
        T += 1  # pad to even so every group is even (pad tiles contribute 0)
    G = choose_group_size(ids, rows, T, N_CORES)
    n_full, rem = divmod(T, G)
    group_sizes = [G] * n_full + ([rem] if rem else [])

    in_maps, bases, b1_nonzero = prepare_core_inputs(
        x, ids, W1, b1, rows, T, group_sizes, N_CORES
    )
    nc = build_nc(T, group_sizes, b1_nonzero)
    bkr = run_bass_kernel_spmd(
        nc, in_maps, list(range(N_CORES)), trace=trace, **spmd_kwargs
    )
    out = merge_outputs(bkr.results, bases, ids, W2, b2, group_sizes, N_CORES, B)
    return out, bkr


def kernel(x, ids, W1, b1, W2, b2):
    return kernel_traced(x, ids, W1, b1, W2, b2, trace=False)[0]


# revision 28
# speedup vs baseline: 1.0011x; 1.0011x over previous
"""Bass/Trainium2 kernel for nn_BagModel (segment_reduce).

Model: h = relu(x @ W1 + b1); per-bag mean of h over sorted ids;
out = means @ W2 + b2.   x:[500000,128] f32, ids:[500000] sorted int64,
W1:[128,256], W2:[256,64], B=10000 bags.

Strategy (8 cores, data-parallel over rows):
- Host: shard rows 62500/core; xT bf16 [128, rows]; group tiles into
  windows spanning < 32 bags (shared group structure across cores, all
  even sizes via padding T to 490); onehot plane [128, T, 32] fp8e4
  (row r of tile t: col rel(r) = bag - window_base).
- Device: per tile h_psum = xT_t.T @ W1 (PE bf16, 256-col stream);
  relu per QUAD of 4 tiles -> SBUF fp8e4 (ACT/DVE alternating 6:5);
  per PAIR one fp8 DoubleRow matmul oh[128,2,32].T @ h2[128,2,256]
  accumulating sums[32,256] in PSUM over the window (dst partition 0
  required by ISA for DoubleRow).
- Window end: copy PSUM->SBUF (ACT/DVE alternating), DMA out
  [NG, 32, 256] f32 partials.
- Host: overlap-add windows into [10000,256], divide by counts,
  means @ W2 + b2 (host GEMM).
"""

import numpy as np
import ml_dtypes
from contextlib import ExitStack

from concourse import bass, tile
from concourse.bass import mybir
from concourse.bass_utils import run_bass_kernel_spmd

N_CORES = 8
N_FULL, D, H, O, B = 500000, 128, 256, 64, 10000
P = 128  # partitions / tile rows
WW = 64  # bag-window width (onehot width)

F32 = mybir.dt.float32
BF16 = mybir.dt.bfloat16
FP8 = mybir.dt.float8e4
BF = ml_dtypes.bfloat16
E4 = ml_dtypes.float8_e4m3

DR = mybir.MatmulPerfMode.DoubleRow
Relu = mybir.ActivationFunctionType.Relu


def build_nc(T, group_sizes, b1_nonzero, relu_pattern=(7, 6), dr_defer=False,
             split_waits=True):
    """One-core program; SPMD-run on all 8 cores with different data.

    T is padded so that every group size is even. Tiles are processed in
    quads (4) + leftover pairs; relu batched per quad.
    """
    NG = len(group_sizes)
    nc = bass.Bass()

    # chunk schedule: ramp up so PE starts early and never outruns DMA
    CH = 32
    chunk_starts = [0, 4, 12, 28]
    while chunk_starts[-1] + CH < T:
        chunk_starts.append(chunk_starts[-1] + CH)
    chunk_of = {}
    for ci, cs in enumerate(chunk_starts):
        ce = chunk_starts[ci + 1] if ci + 1 < len(chunk_starts) else T
        for t in range(cs, ce):
            chunk_of[t] = (ci, cs, ce - cs)

    xt_d = nc.dram_tensor("xt", [P, T * P], BF16, kind="ExternalInput")
    oh_d = nc.dram_tensor("oh", [P, T, WW], FP8, kind="ExternalInput")
    w1_d = nc.dram_tensor("w1", [D, H], BF16, kind="ExternalInput")
    if b1_nonzero:
        b1_d = nc.dram_tensor("b1", [1, H], BF16, kind="ExternalInput")
    out_d = nc.dram_tensor("out_parts", [NG, WW, H], F32, kind="ExternalOutput")

    with tile.TileContext(nc) as tc, ExitStack() as ctx:
        consts = ctx.enter_context(tc.tile_pool(name="consts", bufs=1))
        w1_sb = consts.tile([D, H], BF16)
        if b1_nonzero:
            b1_sb = consts.tile([1, H], BF16)
            ones1_sb = consts.tile([1, P], BF16)
            nc.sync.dma_start(b1_sb[:], b1_d[:])
            nc.gpsimd.memset(ones1_sb[:], 1.0)

        xt_pool = ctx.enter_context(tc.tile_pool(name="xt", bufs=4))
        oh_pool = ctx.enter_context(tc.tile_pool(name="oh", bufs=4))
        h_pool = ctx.enter_context(tc.tile_pool(name="h", bufs=5))
        fl_pool = ctx.enter_context(tc.tile_pool(name="fl", bufs=8))
        hps_pool = ctx.enter_context(
            tc.tile_pool(name="hps", bufs=3, space=bass.MemorySpace.PSUM)
        )
        sps_pool = ctx.enter_context(
            tc.tile_pool(name="sps", bufs=2, space=bass.MemorySpace.PSUM)
        )

        ra, rd = relu_pattern
        rmod = ra + rd

        xt_chunk = oh_chunk = None

        def chunk_loads(t, first=False):
            ci, cs, cw = chunk_of[t]
            if t != cs:
                return
            nonlocal xt_chunk, oh_chunk
            xt_chunk = xt_pool.tile([P, CH * P], BF16)
            nc.sync.dma_start(
                xt_chunk[:, 0 : cw * P], xt_d[:, cs * P : (cs + cw) * P]
            )
            oh_chunk = oh_pool.tile([P, CH, WW], FP8)
            nc.sync.dma_start(oh_chunk[:, 0:cw, :], oh_d[:, cs : cs + cw, :])
            if first:
                # consts on a parallel queue so they overlap the first chunk
                nc.scalar.dma_start(w1_sb[:], w1_d[:])

        def h_matmul(h_ps_slice, t):
            ci, cs, cw = chunk_of[t]
            lhs = xt_chunk[:, (t - cs) * P : (t - cs + 1) * P]
            if b1_nonzero:
                nc.tensor.matmul(h_ps_slice, lhs, w1_sb[:], start=True, stop=False)
                nc.tensor.matmul(
                    h_ps_slice, ones1_sb[:], b1_sb[:], start=False, stop=True
                )
            else:
                nc.tensor.matmul(h_ps_slice, lhs, w1_sb[:], start=True, stop=True)

        def relu(dst, src, qidx):
            if qidx % rmod < ra:
                nc.scalar.activation(dst, src, Relu)
            else:
                nc.vector.tensor_scalar_max(dst, src, 0.0)

        def seg_matmul(sums_ps, h2, hoff, t, oh_c, start, stop):
            ci, cs, cw = chunk_of[t]
            nc.tensor.matmul(
                sums_ps[:, :],
                oh_c[:, t - cs : t - cs + 2, :],
                h2[:, hoff : hoff + 2, :],
                start=start,
                stop=stop,
                perf_mode=DR,
                tile_position=(0, 0),
            )

        t = 0
        qidx = 0  # relu engine selector
        pending_flush = None  # (g, sums_ps) deferred past next window's 1st round
        pending_dr = []  # seg matmuls deferred one round so PE never waits relu

        def emit_flush(relu_was_act):
            nonlocal pending_flush
            if pending_flush is None:
                return
            fg, fps = pending_flush
            pending_flush = None
            sums_sb = fl_pool.tile([WW, H], F32, tag="sums_sb")
            # flush on the engine the current relu is NOT using
            if relu_was_act:
                nc.vector.tensor_copy(sums_sb[:], fps[:])
            else:
                nc.scalar.copy(sums_sb[:], fps[:])
            nc.gpsimd.dma_start(out_d[fg], sums_sb[:])

        def emit_pending_dr():
            for args in pending_dr:
                seg_matmul(*args)
            pending_dr.clear()

        for g, gs in enumerate(group_sizes):
            assert gs % 2 == 0
            sums_ps = sps_pool.tile([WW, H], F32)
            done = 0
            while done < gs:
                qw = min(4, gs - done)  # 4 or 2 tiles this round
                chunk_loads(t, first=(t == 0))
                # tiles of one round never straddle a chunk: starts stay even
                # and chunk starts are multiples of 4
                if t + qw - 1 >= chunk_of[t][1] + chunk_of[t][2]:
                    qw = 2  # don't cross into the next chunk with a quad
                h_ps = hps_pool.tile([P, 4, H], F32, tag="hps")
                for c in range(qw):
                    h_matmul(h_ps[:, c, :], t + c)
                h2 = h_pool.tile([P, 4, H], FP8, tag="h2")
                relu(h2[:, 0:qw, :], h_ps[:, 0:qw, :], qidx)
                emit_pending_dr()  # prior round's seg MMs: relu already landed
                emit_flush(qidx % rmod < ra)  # prior window flush, other engine
                qidx += 1
                oh_c = oh_chunk  # bind current chunk tile for deferred emission
                for c in range(0, qw, 2):
                    pending_dr.append(
                        (
                            sums_ps,
                            h2,
                            c,
                            t + c,
                            oh_c,
                            done + c == 0,
                            done + c + 2 == gs,
                        )
                    )
                if not dr_defer:
                    emit_pending_dr()
                done += qw
                t += qw
            pending_flush = (g, sums_ps)
        emit_pending_dr()
        emit_flush(True)

    if split_waits:
        _split_excess_waits(nc)
    return nc


# walrus codegen rejects instructions whose inline sync-wait list exceeds the
# ISA struct's slots. Move excess waits to standalone EventSemaphore ops on the
# same engine right before the instruction — same-engine FIFO keeps semantics.
_WAIT_LIMITS = {
    "InstTensorTensor": 1,
    "InstTensorScalarPtr": 1,
    "InstTensorScalar": 1,
    "InstTensorCopy": 1,
    "InstTensorReduce": 1,
    "InstCopy": 1,
    "InstActivation": 1,
    "InstMatmult": 1,
    "InstLdweights": 1,
    "InstMemset": 1,
    "InstDMACopy": 1,
    "InstDrain": 1,
    "InstNoOp": 1,
    "InstEventSemaphore": 1,
}


def _split_excess_waits(nc):
    for bb in nc.main_func.blocks:
        new_list = []
        for ins in bb.instructions:
            limit = _WAIT_LIMITS.get(type(ins).__name__)
            si = ins.sync_info
            if limit is not None and si is not None and len(si.on_wait) > limit:
                waits = list(si.on_wait)
                excess, keep = waits[: len(waits) - limit], waits[len(waits) - limit :]
                for w in excess:
                    ev = mybir.InstEventSemaphore(
                        name=nc.get_next_instruction_name(),
                        engine=ins.engine,
                        ins=[],
                        outs=[],
                        sync_info=mybir.SyncInfo(on_wait=[w], on_update=[]),
                    )
                    new_list.append(ev)
                ins.sync_info = mybir.SyncInfo(on_wait=keep, on_update=list(si.on_update))
            new_list.append(ins)
        bb.instructions[:] = new_list


def choose_group_size(ids, rows_per_core, T, n_cores):
    """Largest even G (tiles/group) s.t. every group's bag span < WW on
    every core. T is the padded tile count; padding rows are ignored."""
    for G in (26, 24, 22, 20, 18, 16, 14, 12, 10, 8, 6, 4, 2):
        ok = True
        for k in range(n_cores):
            ids_k = ids[k * rows_per_core : (k + 1) * rows_per_core]
            g = 0
            while g * G < T and ok:
                s = g * G * P
                e = min((g * G + G) * P, rows_per_core)
                if s < rows_per_core:
                    if ids_k[e - 1] - ids_k[s] >= WW:
                        ok = False
                g += 1
            if not ok:
                break
        if ok:
            return G
    raise ValueError("no group size satisfies bag-span < WW")


def prepare_core_inputs(x, ids, W1, b1, rows_per_core, T, group_sizes, n_cores):
    """Returns (in_maps, bases[n_cores, NG], b1_nonzero)."""
    NG = len(group_sizes)
    rpad = T * P
    w1_bf = np.ascontiguousarray(W1.astype(BF))
    b1_nonzero = bool(np.any(b1))

    in_maps = []
    bases = np.zeros((n_cores, NG), np.int64)
    for k in range(n_cores):
        ids_k = ids[k * rows_per_core : (k + 1) * rows_per_core]
        x_k = x[k * rows_per_core : (k + 1) * rows_per_core]
        rel = np.full(rpad, -1, np.int64)
        t0 = 0
        for g, gs in enumerate(group_sizes):
            s = t0 * P
            e = min(s + gs * P, rows_per_core)
            base = int(ids_k[min(s, rows_per_core - 1)])
            bases[k, g] = base
            if s < rows_per_core:
                r = ids_k[s:e].astype(np.int64) - base
                assert r.min() >= 0 and r.max() < WW, (
                    f"bag span violation core {k} group {g}: {r.min()}..{r.max()}"
                )
                rel[s:e] = r
            t0 += gs
        # onehot plane [P, T, WW] fp8: row r=(t,p) sets col rel[r]
        oh = np.zeros((P, T, WW), E4)
        rr = np.arange(rpad)
        valid = rel >= 0
        oh[rr[valid] % P, rr[valid] // P, rel[valid]] = 1
        xt = np.zeros((P, rpad), BF)
        xt[:, :rows_per_core] = x_k.astype(BF).T
        m = {"xt": xt, "oh": oh, "w1": w1_bf}
        if b1_nonzero:
            m["b1"] = np.ascontiguousarray(b1.astype(BF).reshape(1, H))
        in_maps.append(m)
    return in_maps, bases, b1_nonzero


def merge_outputs(results, bases, ids, W2, b2, group_sizes, n_cores, num_bags):
    NG = len(group_sizes)
    acc = np.zeros((num_bags + WW, H), np.float32)
    for k in range(n_cores):
        parts = np.asarray(results[k]["out_parts"], np.float32)  # [NG, WW, H]
        for g in range(NG):
            acc[bases[k, g] : bases[k, g] + WW] += parts[g]
    counts = np.bincount(ids.astype(np.int64), minlength=num_bags)[:num_bags]
    means = acc[:num_bags] / np.maximum(counts, 1.0)[:, None]
    out = means @ W2.astype(np.float32) + b2.astype(np.float32)
    return out.astype(np.float32)


def kernel_traced(x, ids, W1, b1, W2, b2, trace=False, **spmd_kwargs):
    x = np.asarray(x)
    ids = np.asarray(ids).astype(np.int64)
    W1 = np.asarray(W1)
    b1 = np.asarray(b1)
    W2 = np.asarray(W2)
    b2 = np.asarray(b2)

    rows = N_FULL // N_CORES
    T = (rows + P - 1) // P
    if T % 2:
        T += 1  # pad to even so every group is even (pad tiles contribute 0)
    G = choose_group_size(ids, rows, T, N_CORES)
    n_full, rem = divmod(T, G)
    group_sizes = [G] * n_full + ([rem] if rem else [])

    in_maps, bases, b1_nonzero = prepare_core_inputs(
        x, ids, W1, b1, rows, T, group_sizes, N_CORES
    )
    nc = build_nc(T, group_sizes, b1_nonzero)
    bkr = run_bass_kernel_spmd(
        nc, in_maps, list(range(N_CORES)), trace=trace, **spmd_kwargs
    )
    out = merge_outputs(bkr.results, bases, ids, W2, b2, group_sizes, N_CORES, B)
    return out, bkr


def kernel(x, ids, W1, b1, W2, b2):
    return kernel_traced(x, ids, W1, b1, W2, b2, trace=False)[0]


# revision 43
# speedup vs baseline: 1.1864x; 1.1851x over previous
"""Bass/Trainium2 kernel for nn_BagModel (segment_reduce).

Model: h = relu(x @ W1 + b1); per-bag mean of h over sorted ids;
out = means @ W2 + b2.   x:[500000,128] f32, ids:[500000] sorted int64,
W1:[128,256], W2:[256,64], B=10000 bags.

Strategy (8 cores, data-parallel over rows):
- Host: shard rows 62500/core; xT bf16 [128, rows]; group tiles into
  windows spanning < 32 bags (shared group structure across cores, all
  even sizes via padding T to 490); onehot plane [128, T, 32] fp8e4
  (row r of tile t: col rel(r) = bag - window_base).
- Device: per tile h_psum = xT_t.T @ W1 (PE bf16, 256-col stream);
  relu per QUAD of 4 tiles -> SBUF fp8e4 (ACT/DVE alternating 6:5);
  per PAIR one fp8 DoubleRow matmul oh[128,2,32].T @ h2[128,2,256]
  accumulating sums[32,256] in PSUM over the window (dst partition 0
  required by ISA for DoubleRow).
- Window end: copy PSUM->SBUF (ACT/DVE alternating), DMA out
  [NG, 32, 256] f32 partials.
- Host: overlap-add windows into [10000,256], divide by counts,
  means @ W2 + b2 (host GEMM).
"""

import numpy as np
import ml_dtypes
from contextlib import ExitStack

from concourse import bass, tile
from concourse.bass import mybir
from concourse.bass_utils import run_bass_kernel_spmd

N_CORES = 8
N_FULL, D, H, O, B = 500000, 128, 256, 64, 10000
P = 128  # partitions / tile rows
WW = 128  # bag-window width (onehot width)

F32 = mybir.dt.float32
BF16 = mybir.dt.bfloat16
FP8 = mybir.dt.float8e4
BF = ml_dtypes.bfloat16
E4 = ml_dtypes.float8_e4m3

DR = mybir.MatmulPerfMode.DoubleRow
Relu = mybir.ActivationFunctionType.Relu


def build_nc(T, group_sizes, b1_nonzero, relu_pattern=(1, 1), dr_defer=False,
             chunk=32, hps_bufs=3, h_bufs=5, sps_bufs=2, xt_bufs=4, ww=WW,
             ramp=(0, 4, 12, 28), round_tiles=4, relu_split=False,
             split_waits=True):
    """One-core program; SPMD-run on all 8 cores with different data.

    T is padded so that every group size is even. Tiles are processed in
    quads (4) + leftover pairs; relu batched per quad.
    """
    NG = len(group_sizes)
    nc = bass.Bass()

    # chunk schedule: ramp up so PE starts early and never outruns DMA
    CH = chunk
    chunk_starts = list(ramp)
    while chunk_starts[-1] + CH < T:
        chunk_starts.append(chunk_starts[-1] + CH)
    chunk_of = {}
    for ci, cs in enumerate(chunk_starts):
        ce = chunk_starts[ci + 1] if ci + 1 < len(chunk_starts) else T
        for t in range(cs, ce):
            chunk_of[t] = (ci, cs, ce - cs)

    xt_d = nc.dram_tensor("xt", [P, T * P], BF16, kind="ExternalInput")
    oh_d = nc.dram_tensor("oh", [P, T, ww], FP8, kind="ExternalInput")
    w1_d = nc.dram_tensor("w1", [D, H], BF16, kind="ExternalInput")
    if b1_nonzero:
        b1_d = nc.dram_tensor("b1", [1, H], BF16, kind="ExternalInput")
    out_d = nc.dram_tensor("out_parts", [NG, ww, H], F32, kind="ExternalOutput")

    with tile.TileContext(nc) as tc, ExitStack() as ctx:
        consts = ctx.enter_context(tc.tile_pool(name="consts", bufs=1))
        w1_sb = consts.tile([D, H], BF16)
        if b1_nonzero:
            b1_sb = consts.tile([1, H], BF16)
            ones1_sb = consts.tile([1, P], BF16)
            nc.sync.dma_start(b1_sb[:], b1_d[:])
            nc.gpsimd.memset(ones1_sb[:], 1.0)

        xt_pool = ctx.enter_context(tc.tile_pool(name="xt", bufs=xt_bufs))
        oh_pool = ctx.enter_context(tc.tile_pool(name="oh", bufs=xt_bufs))
        h_pool = ctx.enter_context(tc.tile_pool(name="h", bufs=h_bufs))
        fl_pool = ctx.enter_context(tc.tile_pool(name="fl", bufs=8))
        hps_pool = ctx.enter_context(
            tc.tile_pool(name="hps", bufs=hps_bufs, space=bass.MemorySpace.PSUM)
        )
        sps_pool = ctx.enter_context(
            tc.tile_pool(name="sps", bufs=sps_bufs, space=bass.MemorySpace.PSUM)
        )

        ra, rd = relu_pattern
        rmod = ra + rd

        xt_chunk = oh_chunk = None

        def chunk_loads(t, first=False):
            ci, cs, cw = chunk_of[t]
            if t != cs:
                return
            nonlocal xt_chunk, oh_chunk
            xt_chunk = xt_pool.tile([P, CH * P], BF16)
            nc.sync.dma_start(
                xt_chunk[:, 0 : cw * P], xt_d[:, cs * P : (cs + cw) * P]
            )
            oh_chunk = oh_pool.tile([P, CH, ww], FP8)
            nc.sync.dma_start(oh_chunk[:, 0:cw, :], oh_d[:, cs : cs + cw, :])
            if first:
                # consts on a parallel queue so they overlap the first chunk
                nc.scalar.dma_start(w1_sb[:], w1_d[:])

        def h_matmul(h_ps_slice, t):
            ci, cs, cw = chunk_of[t]
            lhs = xt_chunk[:, (t - cs) * P : (t - cs + 1) * P]
            if b1_nonzero:
                nc.tensor.matmul(h_ps_slice, lhs, w1_sb[:], start=True, stop=False)
                nc.tensor.matmul(
                    h_ps_slice, ones1_sb[:], b1_sb[:], start=False, stop=True
                )
            else:
                nc.tensor.matmul(h_ps_slice, lhs, w1_sb[:], start=True, stop=True)

        def relu(dst, src, qidx, qw):
            if relu_split and qw >= 2:
                hw_ = qw // 2
                nc.scalar.activation(dst[:, 0:hw_, :], src[:, 0:hw_, :], Relu)
                nc.vector.tensor_scalar_max(
                    dst[:, hw_:qw, :], src[:, hw_:qw, :], 0.0
                )
            elif qidx % rmod < ra:
                nc.scalar.activation(dst, src, Relu)
            else:
                nc.vector.tensor_scalar_max(dst, src, 0.0)

        def seg_matmul(sums_ps, h2, hoff, t, oh_c, start, stop):
            ci, cs, cw = chunk_of[t]
            nc.tensor.matmul(
                sums_ps[:, :],
                oh_c[:, t - cs : t - cs + 2, :],
                h2[:, hoff : hoff + 2, :],
                start=start,
                stop=stop,
                perf_mode=DR,
                tile_position=(0, 0),
            )

        t = 0
        qidx = 0  # relu engine selector
        pending_flush = None  # (g, sums_ps) deferred past next window's 1st round
        pending_dr = []  # seg matmuls deferred one round so PE never waits relu

        def emit_flush(relu_was_act):
            nonlocal pending_flush
            if pending_flush is None:
                return
            fg, fps = pending_flush
            pending_flush = None
            sums_sb = fl_pool.tile([ww, H], F32, tag="sums_sb")
            # flush on the engine the current relu is NOT using
            if relu_was_act:
                nc.vector.tensor_copy(sums_sb[:], fps[:])
            else:
                nc.scalar.copy(sums_sb[:], fps[:])
            nc.gpsimd.dma_start(out_d[fg], sums_sb[:])

        def emit_pending_dr():
            for args in pending_dr:
                seg_matmul(*args)
            pending_dr.clear()

        for g, gs in enumerate(group_sizes):
            assert gs % 2 == 0
            sums_ps = sps_pool.tile([ww, H], F32)
            done = 0
            while done < gs:
                qw = min(round_tiles, gs - done)  # tiles this round (even)
                chunk_loads(t, first=(t == 0))
                # clamp so a round never straddles a chunk boundary
                cend = chunk_of[t][1] + chunk_of[t][2]
                if t + qw > cend:
                    qw = cend - t
                h_ps = hps_pool.tile([P, round_tiles, H], F32, tag="hps")
                for c in range(qw):
                    h_matmul(h_ps[:, c, :], t + c)
                h2 = h_pool.tile([P, round_tiles, H], FP8, tag="h2")
                relu(h2[:, 0:qw, :], h_ps[:, 0:qw, :], qidx, qw)
                emit_pending_dr()  # prior round's seg MMs: relu already landed
                emit_flush(qidx % rmod < ra)  # prior window flush, other engine
                qidx += 1
                oh_c = oh_chunk  # bind current chunk tile for deferred emission
                for c in range(0, qw, 2):
                    pending_dr.append(
                        (
                            sums_ps,
                            h2,
                            c,
                            t + c,
                            oh_c,
                            done + c == 0,
                            done + c + 2 == gs,
                        )
                    )
                if not dr_defer:
                    emit_pending_dr()
                done += qw
                t += qw
            pending_flush = (g, sums_ps)
        emit_pending_dr()
        emit_flush(True)

    if split_waits:
        _split_excess_waits(nc)
    return nc


# walrus codegen rejects instructions whose inline sync-wait list exceeds the
# ISA struct's slots. Move excess waits to standalone EventSemaphore ops on the
# same engine right before the instruction — same-engine FIFO keeps semantics.
_WAIT_LIMITS = {
    "InstTensorTensor": 1,
    "InstTensorScalarPtr": 1,
    "InstTensorScalar": 1,
    "InstTensorCopy": 1,
    "InstTensorReduce": 1,
    "InstCopy": 1,
    "InstActivation": 1,
    "InstMatmult": 1,
    "InstLdweights": 1,
    "InstMemset": 1,
    "InstDMACopy": 1,
    "InstDrain": 1,
    "InstNoOp": 1,
    "InstEventSemaphore": 1,
}


def _split_excess_waits(nc):
    for bb in nc.main_func.blocks:
        new_list = []
        for ins in bb.instructions:
            limit = _WAIT_LIMITS.get(type(ins).__name__)
            si = ins.sync_info
            if limit is not None and si is not None and len(si.on_wait) > limit:
                waits = list(si.on_wait)
                excess, keep = waits[: len(waits) - limit], waits[len(waits) - limit :]
                for w in excess:
                    ev = mybir.InstEventSemaphore(
                        name=nc.get_next_instruction_name(),
                        engine=ins.engine,
                        ins=[],
                        outs=[],
                        sync_info=mybir.SyncInfo(on_wait=[w], on_update=[]),
                    )
                    new_list.append(ev)
                ins.sync_info = mybir.SyncInfo(on_wait=keep, on_update=list(si.on_update))
            new_list.append(ins)
        bb.instructions[:] = new_list


def choose_group_size(ids, rows_per_core, T, n_cores, ww=WW):
    """Largest even G (tiles/group) s.t. every group's bag span < WW on
    every core. T is the padded tile count; padding rows are ignored."""
    for G in (48, 44, 40, 36, 32, 28, 26, 24, 22, 20, 18, 16, 14, 12, 10, 8,
              6, 4, 2):
        ok = True
        for k in range(n_cores):
            ids_k = ids[k * rows_per_core : (k + 1) * rows_per_core]
            g = 0
            while g * G < T and ok:
                s = g * G * P
                e = min((g * G + G) * P, rows_per_core)
                if s < rows_per_core:
                    if ids_k[e - 1] - ids_k[s] >= ww:
                        ok = False
                g += 1
            if not ok:
                break
        if ok:
            return G
    raise ValueError("no group size satisfies bag-span < ww")


def prepare_core_inputs(x, ids, W1, b1, rows_per_core, T, group_sizes, n_cores, ww=WW):
    """Returns (in_maps, bases[n_cores, NG], b1_nonzero)."""
    NG = len(group_sizes)
    rpad = T * P
    w1_bf = np.ascontiguousarray(W1.astype(BF))
    b1_nonzero = bool(np.any(b1))

    in_maps = []
    bases = np.zeros((n_cores, NG), np.int64)
    for k in range(n_cores):
        ids_k = ids[k * rows_per_core : (k + 1) * rows_per_core]
        x_k = x[k * rows_per_core : (k + 1) * rows_per_core]
        rel = np.full(rpad, -1, np.int64)
        t0 = 0
        for g, gs in enumerate(group_sizes):
            s = t0 * P
            e = min(s + gs * P, rows_per_core)
            base = int(ids_k[min(s, rows_per_core - 1)])
            bases[k, g] = base
            if s < rows_per_core:
                r = ids_k[s:e].astype(np.int64) - base
                assert r.min() >= 0 and r.max() < ww, (
                    f"bag span violation core {k} group {g}: {r.min()}..{r.max()}"
                )
                rel[s:e] = r
            t0 += gs
        # onehot plane [P, T, WW] fp8: row r=(t,p) sets col rel[r]
        oh = np.zeros((P, T, ww), E4)
        rr = np.arange(rpad)
        valid = rel >= 0
        oh[rr[valid] % P, rr[valid] // P, rel[valid]] = 1
        xt = np.zeros((P, rpad), BF)
        xt[:, :rows_per_core] = x_k.astype(BF).T
        m = {"xt": xt, "oh": oh, "w1": w1_bf}
        if b1_nonzero:
            m["b1"] = np.ascontiguousarray(b1.astype(BF).reshape(1, H))
        in_maps.append(m)
    return in_maps, bases, b1_nonzero


def merge_outputs(results, bases, ids, W2, b2, group_sizes, n_cores, num_bags, ww=WW):
    NG = len(group_sizes)
    acc = np.zeros((num_bags + ww, H), np.float32)
    for k in range(n_cores):
        parts = np.asarray(results[k]["out_parts"], np.float32)  # [NG, WW, H]
        for g in range(NG):
            acc[bases[k, g] : bases[k, g] + ww] += parts[g]
    counts = np.bincount(ids.astype(np.int64), minlength=num_bags)[:num_bags]
    means = acc[:num_bags] / np.maximum(counts, 1.0)[:, None]
    out = means @ W2.astype(np.float32) + b2.astype(np.float32)
    return out.astype(np.float32)


def kernel_traced(x, ids, W1, b1, W2, b2, trace=False, **spmd_kwargs):
    x = np.asarray(x)
    ids = np.asarray(ids).astype(np.int64)
    W1 = np.asarray(W1)
    b1 = np.asarray(b1)
    W2 = np.asarray(W2)
    b2 = np.asarray(b2)

    rows = N_FULL // N_CORES
    T = (rows + P - 1) // P
    if T % 2:
        T += 1  # pad to even so every group is even (pad tiles contribute 0)
    G = choose_group_size(ids, rows, T, N_CORES)
    n_full, rem = divmod(T, G)
    group_sizes = [G] * n_full + ([rem] if rem else [])

    in_maps, bases, b1_nonzero = prepare_core_inputs(
        x, ids, W1, b1, rows, T, group_sizes, N_CORES
    )
    nc = build_nc(T, group_sizes, b1_nonzero)
    bkr = run_bass_kernel_spmd(
        nc, in_maps, list(range(N_CORES)), trace=trace, **spmd_kwargs
    )
    out = merge_outputs(bkr.results, bases, ids, W2, b2, group_sizes, N_CORES, B)
    return out, bkr


def kernel(x, ids, W1, b1, W2, b2):
    return kernel_traced(x, ids, W1, b1, W2, b2, trace=False)[0]


# revision 44
# speedup vs baseline: 1.4023x; 1.1819x over previous
"""Bass/Trainium2 kernel for nn_BagModel (segment_reduce).

Model: h = relu(x @ W1 + b1); per-bag mean of h over sorted ids;
out = means @ W2 + b2.   x:[500000,128] f32, ids:[500000] sorted int64,
W1:[128,256], W2:[256,64], B=10000 bags.

Strategy (8 cores, data-parallel over rows; ~116us on a fast-clock core
vs 224us for the onehot-from-host bf16 baseline):
- Host: shard rows 62500/core; xT bf16 [128, rows] (T=490 tiles, padded
  even); group tiles into windows spanning < 128 bags (G=44 tiles,
  shared group structure across cores, all sizes even); onehot plane
  [128, T, 128] fp8e4 (row r of tile t: col = bag - window_base).
- Device, per round of 4 tiles: h_psum = xT_t.T @ W1 (PE bf16, 256-col
  stream, one LDW+MM per tile); relu for the whole round -> SBUF fp8e4
  (ACT/DVE strictly alternating rounds); per PAIR one fp8 DoubleRow
  matmul oh[128,2,128].T @ h2[128,2,256] contracting 256 rows in one
  instruction, accumulating sums[128,256] in PSUM across the window
  (DoubleRow requires dst partition 0, hence one window per PSUM tile).
- Window end: copy PSUM->SBUF on the engine the current relu is NOT
  using, deferred one round to avoid head-of-line blocking; DMA out
  [NG, 128, 256] f32 partials on the gpsimd queue (keeps the sync
  queue's chunk DMAs HOL-free).
- Host: overlap-add windows into [10000,256], divide by counts,
  means @ W2 + b2 (host GEMM).
Key perf facts learned on hw: DoubleRow streams 1 cyc/row (the 2x is
the doubled contraction); PSUM = 8 banks, hps 3x2 + sps 2x1 fills it;
pipeline depth (hps_bufs=3, fl_bufs=8) matters more than anything; DMA
descriptor-gen costs ~600ns/dma_start on the issuing engine's queue.
"""

import numpy as np
import ml_dtypes
from contextlib import ExitStack

from concourse import bass, tile
from concourse.bass import mybir
from concourse.bass_utils import run_bass_kernel_spmd

N_CORES = 8
N_FULL, D, H, O, B = 500000, 128, 256, 64, 10000
P = 128  # partitions / tile rows
WW = 128  # bag-window width (onehot width)

F32 = mybir.dt.float32
BF16 = mybir.dt.bfloat16
FP8 = mybir.dt.float8e4
BF = ml_dtypes.bfloat16
E4 = ml_dtypes.float8_e4m3

DR = mybir.MatmulPerfMode.DoubleRow
Relu = mybir.ActivationFunctionType.Relu


def build_nc(T, group_sizes, b1_nonzero, relu_pattern=(1, 1), dr_defer=False,
             chunk=32, hps_bufs=3, h_bufs=5, sps_bufs=2, xt_bufs=4, ww=WW,
             ramp=(0, 4, 12, 28), round_tiles=4, relu_split=False,
             split_waits=True):
    """One-core program; SPMD-run on all 8 cores with different data.

    T is padded so that every group size is even. Tiles are processed in
    quads (4) + leftover pairs; relu batched per quad.
    """
    NG = len(group_sizes)
    nc = bass.Bass()

    # chunk schedule: ramp up so PE starts early and never outruns DMA
    CH = chunk
    chunk_starts = list(ramp)
    while chunk_starts[-1] + CH < T:
        chunk_starts.append(chunk_starts[-1] + CH)
    chunk_of = {}
    for ci, cs in enumerate(chunk_starts):
        ce = chunk_starts[ci + 1] if ci + 1 < len(chunk_starts) else T
        for t in range(cs, ce):
            chunk_of[t] = (ci, cs, ce - cs)

    xt_d = nc.dram_tensor("xt", [P, T * P], BF16, kind="ExternalInput")
    oh_d = nc.dram_tensor("oh", [P, T, ww], FP8, kind="ExternalInput")
    w1_d = nc.dram_tensor("w1", [D, H], BF16, kind="ExternalInput")
    if b1_nonzero:
        b1_d = nc.dram_tensor("b1", [1, H], BF16, kind="ExternalInput")
    out_d = nc.dram_tensor("out_parts", [NG, ww, H], F32, kind="ExternalOutput")

    with tile.TileContext(nc) as tc, ExitStack() as ctx:
        consts = ctx.enter_context(tc.tile_pool(name="consts", bufs=1))
        w1_sb = consts.tile([D, H], BF16)
        if b1_nonzero:
            b1_sb = consts.tile([1, H], BF16)
            ones1_sb = consts.tile([1, P], BF16)
            nc.sync.dma_start(b1_sb[:], b1_d[:])
            nc.gpsimd.memset(ones1_sb[:], 1.0)

        xt_pool = ctx.enter_context(tc.tile_pool(name="xt", bufs=xt_bufs))
        oh_pool = ctx.enter_context(tc.tile_pool(name="oh", bufs=xt_bufs))
        h_pool = ctx.enter_context(tc.tile_pool(name="h", bufs=h_bufs))
        fl_pool = ctx.enter_context(tc.tile_pool(name="fl", bufs=8))
        hps_pool = ctx.enter_context(
            tc.tile_pool(name="hps", bufs=hps_bufs, space=bass.MemorySpace.PSUM)
        )
        sps_pool = ctx.enter_context(
            tc.tile_pool(name="sps", bufs=sps_bufs, space=bass.MemorySpace.PSUM)
        )

        ra, rd = relu_pattern
        rmod = ra + rd

        xt_chunk = oh_chunk = None

        def chunk_loads(t, first=False):
            ci, cs, cw = chunk_of[t]
            if t != cs:
                return
            nonlocal xt_chunk, oh_chunk
            xt_chunk = xt_pool.tile([P, CH * P], BF16)
            nc.sync.dma_start(
                xt_chunk[:, 0 : cw * P], xt_d[:, cs * P : (cs + cw) * P]
            )
            oh_chunk = oh_pool.tile([P, CH, ww], FP8)
            nc.sync.dma_start(oh_chunk[:, 0:cw, :], oh_d[:, cs : cs + cw, :])
            if first:
                # consts on a parallel queue so they overlap the first chunk
                nc.scalar.dma_start(w1_sb[:], w1_d[:])

        def h_matmul(h_ps_slice, t):
            ci, cs, cw = chunk_of[t]
            lhs = xt_chunk[:, (t - cs) * P : (t - cs + 1) * P]
            if b1_nonzero:
                nc.tensor.matmul(h_ps_slice, lhs, w1_sb[:], start=True, stop=False)
                nc.tensor.matmul(
                    h_ps_slice, ones1_sb[:], b1_sb[:], start=False, stop=True
                )
            else:
                nc.tensor.matmul(h_ps_slice, lhs, w1_sb[:], start=True, stop=True)

        def relu(dst, src, qidx, qw):
            if relu_split and qw >= 2:
                hw_ = qw // 2
                nc.scalar.activation(dst[:, 0:hw_, :], src[:, 0:hw_, :], Relu)
                nc.vector.tensor_scalar_max(
                    dst[:, hw_:qw, :], src[:, hw_:qw, :], 0.0
                )
            elif qidx % rmod < ra:
                nc.scalar.activation(dst, src, Relu)
            else:
                nc.vector.tensor_scalar_max(dst, src, 0.0)

        def seg_matmul(sums_ps, h2, hoff, t, oh_c, start, stop):
            ci, cs, cw = chunk_of[t]
            nc.tensor.matmul(
                sums_ps[:, :],
                oh_c[:, t - cs : t - cs + 2, :],
                h2[:, hoff : hoff + 2, :],
                start=start,
                stop=stop,
                perf_mode=DR,
                tile_position=(0, 0),
            )

        t = 0
        qidx = 0  # relu engine selector
        pending_flush = None  # (g, sums_ps) deferred past next window's 1st round
        pending_dr = []  # seg matmuls deferred one round so PE never waits relu

        def emit_flush(relu_was_act):
            nonlocal pending_flush
            if pending_flush is None:
                return
            fg, fps = pending_flush
            pending_flush = None
            sums_sb = fl_pool.tile([ww, H], F32, tag="sums_sb")
            # flush on the engine the current relu is NOT using
            if relu_was_act:
                nc.vector.tensor_copy(sums_sb[:], fps[:])
            else:
                nc.scalar.copy(sums_sb[:], fps[:])
            nc.gpsimd.dma_start(out_d[fg], sums_sb[:])

        def emit_pending_dr():
            for args in pending_dr:
                seg_matmul(*args)
            pending_dr.clear()

        for g, gs in enumerate(group_sizes):
            assert gs % 2 == 0
            sums_ps = sps_pool.tile([ww, H], F32)
            done = 0
            while done < gs:
                qw = min(round_tiles, gs - done)  # tiles this round (even)
                chunk_loads(t, first=(t == 0))
                # clamp so a round never straddles a chunk boundary
                cend = chunk_of[t][1] + chunk_of[t][2]
                if t + qw > cend:
                    qw = cend - t
                h_ps = hps_pool.tile([P, round_tiles, H], F32, tag="hps")
                for c in range(qw):
                    h_matmul(h_ps[:, c, :], t + c)
                h2 = h_pool.tile([P, round_tiles, H], FP8, tag="h2")
                relu(h2[:, 0:qw, :], h_ps[:, 0:qw, :], qidx, qw)
                emit_pending_dr()  # prior round's seg MMs: relu already landed
                emit_flush(qidx % rmod < ra)  # prior window flush, other engine
                qidx += 1
                oh_c = oh_chunk  # bind current chunk tile for deferred emission
                for c in range(0, qw, 2):
                    pending_dr.append(
                        (
                            sums_ps,
                            h2,
                            c,
                            t + c,
                            oh_c,
                            done + c == 0,
                            done + c + 2 == gs,
                        )
                    )
                if not dr_defer:
                    emit_pending_dr()
                done += qw
                t += qw
            pending_flush = (g, sums_ps)
        emit_pending_dr()
        emit_flush(True)

    if split_waits:
        _split_excess_waits(nc)
    return nc


# walrus codegen rejects instructions whose inline sync-wait list exceeds the
# ISA struct's slots. Move excess waits to standalone EventSemaphore ops on the
# same engine right before the instruction — same-engine FIFO keeps semantics.
_WAIT_LIMITS = {
    "InstTensorTensor": 1,
    "InstTensorScalarPtr": 1,
    "InstTensorScalar": 1,
    "InstTensorCopy": 1,
    "InstTensorReduce": 1,
    "InstCopy": 1,
    "InstActivation": 1,
    "InstMatmult": 1,
    "InstLdweights": 1,
    "InstMemset": 1,
    "InstDMACopy": 1,
    "InstDrain": 1,
    "InstNoOp": 1,
    "InstEventSemaphore": 1,
}


def _split_excess_waits(nc):
    for bb in nc.main_func.blocks:
        new_list = []
        for ins in bb.instructions:
            limit = _WAIT_LIMITS.get(type(ins).__name__)
            si = ins.sync_info
            if limit is not None and si is not None and len(si.on_wait) > limit:
                waits = list(si.on_wait)
                excess, keep = waits[: len(waits) - limit], waits[len(waits) - limit :]
                for w in excess:
                    ev = mybir.InstEventSemaphore(
                        name=nc.get_next_instruction_name(),
                        engine=ins.engine,
                        ins=[],
                        outs=[],
                        sync_info=mybir.SyncInfo(on_wait=[w], on_update=[]),
                    )
                    new_list.append(ev)
                ins.sync_info = mybir.SyncInfo(on_wait=keep, on_update=list(si.on_update))
            new_list.append(ins)
        bb.instructions[:] = new_list


def choose_group_size(ids, rows_per_core, T, n_cores, ww=WW):
    """Largest even G (tiles/group) s.t. every group's bag span < WW on
    every core. T is the padded tile count; padding rows are ignored."""
    for G in (48, 44, 40, 36, 32, 28, 26, 24, 22, 20, 18, 16, 14, 12, 10, 8,
              6, 4, 2):
        ok = True
        for k in range(n_cores):
            ids_k = ids[k * rows_per_core : (k + 1) * rows_per_core]
            g = 0
            while g * G < T and ok:
                s = g * G * P
                e = min((g * G + G) * P, rows_per_core)
                if s < rows_per_core:
                    if ids_k[e - 1] - ids_k[s] >= ww:
                        ok = False
                g += 1
            if not ok:
                break
        if ok:
            return G
    raise ValueError("no group size satisfies bag-span < ww")


def prepare_core_inputs(x, ids, W1, b1, rows_per_core, T, group_sizes, n_cores, ww=WW):
    """Returns (in_maps, bases[n_cores, NG], b1_nonzero)."""
    NG = len(group_sizes)
    rpad = T * P
    w1_bf = np.ascontiguousarray(W1.astype(BF))
    b1_nonzero = bool(np.any(b1))

    in_maps = []
    bases = np.zeros((n_cores, NG), np.int64)
    for k in range(n_cores):
        ids_k = ids[k * rows_per_core : (k + 1) * rows_per_core]
        x_k = x[k * rows_per_core : (k + 1) * rows_per_core]
        rel = np.full(rpad, -1, np.int64)
        t0 = 0
        for g, gs in enumerate(group_sizes):
            s = t0 * P
            e = min(s + gs * P, rows_per_core)
            base = int(ids_k[min(s, rows_per_core - 1)])
            bases[k, g] = base
            if s < rows_per_core:
                r = ids_k[s:e].astype(np.int64) - base
                assert r.min() >= 0 and r.max() < ww, (
                    f"bag span violation core {k} group {g}: {r.min()}..{r.max()}"
                )
                rel[s:e] = r
            t0 += gs
        # onehot plane [P, T, WW] fp8: row r=(t,p) sets col rel[r]
        oh = np.zeros((P, T, ww), E4)
        rr = np.arange(rpad)
        valid = rel >= 0
        oh[rr[valid] % P, rr[valid] // P, rel[valid]] = 1
        xt = np.zeros((P, rpad), BF)
        xt[:, :rows_per_core] = x_k.astype(BF).T
        m = {"xt": xt, "oh": oh, "w1": w1_bf}
        if b1_nonzero:
            m["b1"] = np.ascontiguousarray(b1.astype(BF).reshape(1, H))
        in_maps.append(m)
    return in_maps, bases, b1_nonzero


def merge_outputs(results, bases, ids, W2, b2, group_sizes, n_cores, num_bags, ww=WW):
    NG = len(group_sizes)
    acc = np.zeros((num_bags + ww, H), np.float32)
    for k in range(n_cores):
        parts = np.asarray(results[k]["out_parts"], np.float32)  # [NG, WW, H]
        for g in range(NG):
            acc[bases[k, g] : bases[k, g] + ww] += parts[g]
    counts = np.bincount(ids.astype(np.int64), minlength=num_bags)[:num_bags]
    means = acc[:num_bags] / np.maximum(counts, 1.0)[:, None]
    out = means @ W2.astype(np.float32) + b2.astype(np.float32)
    return out.astype(np.float32)


def kernel_traced(x, ids, W1, b1, W2, b2, trace=False, **spmd_kwargs):
    x = np.asarray(x)
    ids = np.asarray(ids).astype(np.int64)
    W1 = np.asarray(W1)
    b1 = np.asarray(b1)
    W2 = np.asarray(W2)
    b2 = np.asarray(b2)

    rows = N_FULL // N_CORES
    T = (rows + P - 1) // P
    if T % 2:
        T += 1  # pad to even so every group is even (pad tiles contribute 0)
    G = choose_group_size(ids, rows, T, N_CORES)
    n_full, rem = divmod(T, G)
    group_sizes = [G] * n_full + ([rem] if rem else [])

    in_maps, bases, b1_nonzero = prepare_core_inputs(
        x, ids, W1, b1, rows, T, group_sizes, N_CORES
    )
    nc = build_nc(T, group_sizes, b1_nonzero)
    bkr = run_bass_kernel_spmd(
        nc, in_maps, list(range(N_CORES)), trace=trace, **spmd_kwargs
    )
    out = merge_outputs(bkr.results, bases, ids, W2, b2, group_sizes, N_CORES, B)
    return out, bkr


def kernel(x, ids, W1, b1, W2, b2):
    return kernel_traced(x, ids, W1, b1, W2, b2, trace=False)[0]
